# revision 24
# baseline (speedup 1.0000x reference)
"""Trainium2 Bass kernel for a dense transformer block (pre-LN, causal MHA + FFN).

Sharding (8 NeuronCores): core c = 2*b + g handles sequence b (of B=4) and
half g (of 2): tensor-parallel attention over 8 of 16 heads (partial proj,
pairwise ReduceScatter over {2b, 2b+1}), then token-parallel LN2+FFN over
its 1024 of 2048 tokens. Device kernel works in transposed [C, T] layout;
host transposes in/out.

LayerNorm is folded into the matmuls: for Q^T = Wq^T @ LN(x)^T we accumulate
M = W~^T x plus rank-1 corrections (colsum(W~) (x) -mu + (beta@W) (x) std)
in PSUM, then scale columns by rstd at eviction. Same trick for the FFN
(relu is positively homogeneous, so rstd2 commutes out to the ff2 evict).

Matmul dtypes: fp8e4m3 in DoubleRow mode (0.5 cycles/row, contraction
folded 2x along the free axis) for QKV / scores / LN stats / corrections.
The FFN runs a 3-term fp8 residual decomposition (W_hi@x_hi + W_lo@x_hi +
W_hi@x_lo, each DoubleRow) which is more accurate than bf16 at 0.75x the
bf16 matmul cost. Attention PV / proj stay bf16.
"""
import numpy as np
import ml_dtypes
from contextlib import ExitStack

B, T, C = 4, 2048, 1024
H, HS = 16, 64
F = 4 * C
P = 128
EPS = 1e-5
NCT = C // P        # 8 c-tiles
NCP = NCT // 2      # 4 c-tile pairs (DoubleRow fold)
NFT = F // P        # 32 f-tiles
NFP = NFT // 2      # 16 f-tile pairs
TL = T // 2         # 1024 local tokens
NPAIR = 4           # head-pairs per core
GROUPS = [[0, 1], [2, 3], [4, 5], [6, 7]]
WSCALE = 32.0       # weights are scaled into fp8's normal range; unscaled at
                    # the (already present) evict multiplies
SSCALE = float(HS ** -0.5)

F8 = ml_dtypes.float8_e4m3

_CACHE = {}


def _build(with_collective=True):
    import concourse.tile as tile
    from concourse import bacc, mybir

    f32 = mybir.dt.float32
    bf16 = mybir.dt.bfloat16
    fp8 = mybir.dt.float8e4
    AF = mybir.ActivationFunctionType
    OP = mybir.AluOpType
    DR = mybir.MatmulPerfMode.DoubleRow

    nc = bacc.Bacc("TRN2", target_bir_lowering=False, debug=False, num_devices=8)

    # ---- DRAM I/O ----
    d_xT = nc.dram_tensor("xT", [P, NCT, T], f32, kind="ExternalInput").ap()
    d_xres = nc.dram_tensor("xresT", [NCT, P, TL], f32, kind="ExternalInput").ap()
    d_wqkv = nc.dram_tensor("wqkv", [NPAIR, P, NCP, 2, 3 * P], fp8,
                            kind="ExternalInput").ap()
    d_ccqkv = nc.dram_tensor("ccqkv", [NPAIR, 1, 2, 3 * P], fp8,
                             kind="ExternalInput").ap()
    d_wproj = nc.dram_tensor("wproj", [P, NPAIR, C], bf16, kind="ExternalInput").ap()
    d_bproj = nc.dram_tensor("bproj", [P, NCT], f32, kind="ExternalInput").ap()
    d_w1hl = nc.dram_tensor("w1hl", [8, P, NCP, 2, 1024], fp8,
                            kind="ExternalInput").ap()
    d_ccf = nc.dram_tensor("ccf", [8, 2, 512], fp8, kind="ExternalInput").ap()
    d_w2hl = nc.dram_tensor("w2hl", [NCT, P, NFP, 2, 256], fp8,
                            kind="ExternalInput").ap()
    d_b2 = nc.dram_tensor("b2", [P, NCT], f32, kind="ExternalInput").ap()
    d_m01 = nc.dram_tensor("m01", [P, 4, 512], bf16, kind="ExternalInput").ap()
    d_ident = nc.dram_tensor("ident", [P, P], f32, kind="ExternalInput").ap()
    d_out = nc.dram_tensor("outT", [NCT, P, TL], f32, kind="ExternalOutput").ap()

    with tile.TileContext(nc) as tc, ExitStack() as ctx:
        dram = ctx.enter_context(tc.tile_pool(name="dram", bufs=1, space="DRAM"))
        sa_bounce = [dram.tile([2, NCT // 2, P, TL], f32, name=f"sab{h}")
                     for h in range(2)]
        sa_local = [dram.tile([NCT // 2, P, TL], f32, name=f"sal{h}")
                    for h in range(2)]

        const = ctx.enter_context(tc.tile_pool(name="const", bufs=1))
        # DoubleRow needs >=32 output partitions (walrus codegen limit):
        # stats matmuls produce [32, 512] and read row 0.
        ones8 = const.tile([P, 2, 32], fp8)
        nc.vector.memset(ones8[:], 1.0)
        ident_f32 = const.tile([P, P], f32)
        nc.sync.dma_start(ident_f32[:], d_ident[:])
        ident_bf = const.tile([P, P], bf16)
        nc.vector.tensor_copy(ident_bf[:], ident_f32[:])
        mask4 = const.tile([P, 4, 512], bf16)
        nc.scalar.dma_start(mask4[:], d_m01[:])
        masks = [mask4[:, i, :] for i in range(4)]
        mask_tri8 = const.tile([P, 128], fp8)
        nc.vector.tensor_copy(mask_tri8[:], mask4[:, 0, 0:128])
        onescol = const.tile([1, P], f32)
        nc.vector.memset(onescol[:], 1.0)

        x2_dram = dram.tile([NCT, P, TL], f32)

        # long-lived pools, first-use pinned bottom-up so frees are LIFO.
        abc_pool = ctx.enter_context(tc.tile_pool(name="abc", bufs=1))
        a1bc = [abc_pool.tile([P, 512], f32, name=f"a1bc{ch}", tag=f"a1bc{ch}")
                for ch in range(4)]
        a2bc = [abc_pool.tile([P, 512], f32, name=f"a2bc{ch}", tag=f"a2bc{ch}")
                for ch in range(2)]
        for t_ in a1bc + a2bc:
            nc.vector.memset(t_[:, 0:1], 0.0)  # pin allocation order
        rowr_pool = ctx.enter_context(tc.tile_pool(name="rowr", bufs=1))
        rowr_pin = rowr_pool.tile([1, 1], f32, tag="pin")
        nc.vector.memset(rowr_pin[:], 0.0)
        pattr = ExitStack()
        attT_pool = pattr.enter_context(tc.tile_pool(name="attT", bufs=1))
        attT = [attT_pool.tile([P, T], bf16, name=f"attT{p}", tag=f"attT{p}")
                for p in range(NPAIR)]
        for p in range(NPAIR):
            nc.vector.memset(attT[p][:, 0:1], 0.0)
        pqkv = ExitStack()
        vaug_pool = pqkv.enter_context(tc.tile_pool(name="vaug", bufs=1))
        qq_pool = pqkv.enter_context(tc.tile_pool(name="qq", bufs=1))
        kk_pool = pqkv.enter_context(tc.tile_pool(name="kk", bufs=1))
        v_aug = {}
        for p in range(NPAIR):
            for sp in range(8):
                va = vaug_pool.tile([P, 2, 130], fp8, name=f"va{p}_{sp}",
                                    tag=f"va{p}_{sp}")
                nc.vector.memset(va[:, :, 64:65], 1.0)
                nc.vector.memset(va[:, :, 129:130], 1.0)
                v_aug[(p, sp)] = va
        # fp8 Q/K with zero-padded DoubleRow fold: slot 0 = data, slot 1 = 0
        qq_r = [qq_pool.tile([P, 2, T], fp8, name=f"qq{p}", tag=f"qq{p}")
                for p in range(NPAIR)]
        kk_r = [kk_pool.tile([P, 2, T], fp8, name=f"kk{p}", tag=f"kk{p}")
                for p in range(NPAIR)]
        for p in range(NPAIR):
            nc.gpsimd.memset(qq_r[p][:, 1, :], 0.0)
            nc.gpsimd.memset(kk_r[p][:, 1, :], 0.0)

        # fp8 DR-folded x tiles, persist through QKV
        px8 = ExitStack()
        x8_pool = px8.enter_context(tc.tile_pool(name="x8", bufs=1))
        x8 = {}
        for j in range(NCP):
            for ch in range(4):
                t_ = x8_pool.tile([P, 2, 512], fp8, name=f"x8_{j}_{ch}",
                                  tag=f"x8_{j}_{ch}")
                x8[(j, ch)] = t_

        # =========== Phase 1: LN1 stats (fp8 DoubleRow) ===========
        p1 = ExitStack()
        xc_pool = p1.enter_context(tc.tile_pool(name="xc", bufs=3))
        xsq_pool = p1.enter_context(tc.tile_pool(name="xsq", bufs=5))
        rows1_pool = p1.enter_context(tc.tile_pool(name="rows1", bufs=6))
        stat_ps = p1.enter_context(tc.tile_pool(name="statps", bufs=2, space="PSUM"))
        bcp_ps = p1.enter_context(tc.tile_pool(name="bcpps", bufs=2, space="PSUM"))

        mu_row = rows1_pool.tile([1, T], f32, tag="row")
        ex2_row = rows1_pool.tile([1, T], f32, tag="row")
        var_row = rows1_pool.tile([1, T], f32, tag="row")
        std_row = rows1_pool.tile([1, T], f32, tag="row")
        rstd_row = rows1_pool.tile([1, T], f32, tag="row")
        nm_row = rows1_pool.tile([1, T], f32, tag="row")
        xrow1_8 = rowr_pool.tile([1, 2, T], fp8, tag="xrow8")
        xcs = []
        for ch in range(4):
            xc = xc_pool.tile([P, NCT, 512], f32, tag="xc", name=f"xc{ch}")
            eng = nc.sync if ch % 2 == 0 else nc.scalar
            eng.dma_start(xc[:], d_xT[:, :, ch * 512:(ch + 1) * 512])
            xcs.append(xc)
        for ch in range(4):
            sl = slice(ch * 512, (ch + 1) * 512)
            sx_ps = stat_ps.tile([32, 512], f32, tag="sx")
            sq_ps = stat_ps.tile([32, 512], f32, tag="sq")
            xc = xcs[ch]
            for ci in range(NCT):
                j, i2 = ci // 2, ci % 2
                nc.vector.tensor_copy(x8[(j, ch)][:, i2, :], xc[:, ci, :])
                if i2 == 0:
                    xsq = xsq_pool.tile([P, 2, 512], fp8, tag="xsq")
                if ci % 2 == 0:
                    nc.scalar.activation(xsq[:, i2, :], xc[:, ci, :], AF.Square)
                else:
                    nc.vector.tensor_mul(xsq[:, i2, :], xc[:, ci, :],
                                         xc[:, ci, :])
                if i2 == 1:
                    nc.tensor.matmul(sx_ps[:], ones8[:], x8[(j, ch)][:],
                                     start=(j == 0), stop=(j == NCP - 1),
                                     perf_mode=DR)
                    nc.tensor.matmul(sq_ps[:], ones8[:], xsq[:],
                                     start=(j == 0), stop=(j == NCP - 1),
                                     perf_mode=DR)
            # per-chunk stats math so QKV corrections unblock early
            nc.scalar.mul(mu_row[:, sl], sx_ps[0:1, :], 1.0 / C)
            nc.scalar.mul(ex2_row[:, sl], sq_ps[0:1, :], 1.0 / C)
            nc.vector.tensor_mul(var_row[:, sl], mu_row[:, sl], mu_row[:, sl])
            nc.vector.scalar_tensor_tensor(var_row[:, sl], ex2_row[:, sl], EPS,
                                           var_row[:, sl], OP.add, OP.subtract)
            nc.scalar.activation(std_row[:, sl], var_row[:, sl], AF.Sqrt)
            nc.vector.reciprocal(rstd_row[:, sl], std_row[:, sl])
            nc.scalar.mul(nm_row[:, sl], mu_row[:, sl], -1.0)
            nc.vector.tensor_copy(xrow1_8[0:1, 0, sl], nm_row[:, sl])
            nc.vector.tensor_copy(xrow1_8[0:1, 1, sl], std_row[:, sl])
            bc_ps = bcp_ps.tile([P, 512], f32, tag="bc")
            nc.tensor.matmul(bc_ps[:], onescol[:], rstd_row[:, sl],
                             start=True, stop=True)
            nc.scalar.mul(a1bc[ch][:], bc_ps[:], 1.0 / WSCALE)
        p1.close()

        # ===== Phases 2+3 (interleaved per pair): QKV + attention =====
        pat = ExitStack()
        w_pool = pat.enter_context(tc.tile_pool(name="wqkv", bufs=2))
        cc_pool = pat.enter_context(tc.tile_pool(name="cc", bufs=1))
        ev_pool = pat.enter_context(tc.tile_pool(name="ev", bufs=3))
        e_pool = pat.enter_context(tc.tile_pool(name="epool", bufs=8))
        rec_pool = pat.enter_context(tc.tile_pool(name="rec", bufs=2))
        bcsb_pool = pat.enter_context(tc.tile_pool(name="bcsb", bufs=2))
        mps = pat.enter_context(tc.tile_pool(name="mps", bufs=1, space="PSUM"))

        cc_all = cc_pool.tile([1, NPAIR, 2, 3 * P], fp8, tag="cc")
        nc.gpsimd.dma_start(cc_all[:], d_ccqkv[:])
        mask_tri = masks[0]  # [P, 512]; cols 0:128 hold the (s <= q) triangle

        def qkv_gen(p):
            """QKV for pair p, yielded in small chunks so the attention loop
            of pair p-1 can interleave them into the PE stream as filler."""
            wq_t = w_pool.tile([P, NCP, 2, 3 * P], fp8, tag="w", name=f"wq{p}")
            nc.scalar.dma_start(wq_t[:], d_wqkv[p])
            cc = cc_all[:, p]
            yield
            for ch in range(4):
                sl = slice(ch * 512, (ch + 1) * 512)
                vev = None
                for comp in range(3):
                    csl = slice(comp * P, (comp + 1) * P)
                    acc = mps.tile([P, 512], f32, tag="qkv", name="acc")
                    for j in range(NCP):
                        nc.tensor.matmul(acc[:], wq_t[:, j, :, csl],
                                         x8[(j, ch)][:], start=(j == 0),
                                         stop=False, perf_mode=DR)
                    nc.tensor.matmul(acc[:], cc[:, :, csl], xrow1_8[:, :, sl],
                                     start=False, stop=True, perf_mode=DR)
                    if comp == 0:
                        nc.vector.tensor_mul(qq_r[p][:, 0, sl], acc[:],
                                             a1bc[ch][:])
                    elif comp == 1:
                        nc.vector.tensor_mul(kk_r[p][:, 0, sl], acc[:],
                                             a1bc[ch][:])
                    else:
                        vev = ev_pool.tile([P, 512], bf16, tag="vev", name="vev")
                        nc.vector.tensor_mul(vev[:], acc[:], a1bc[ch][:])
                    yield
                for sti in range(4):
                    st = ch * 4 + sti
                    for hh in range(2):
                        tr = mps.tile([P, 64], bf16, tag="tr", name="tr")
                        nc.tensor.transpose(
                            tr[:],
                            vev[hh * 64:(hh + 1) * 64, sti * 128:(sti + 1) * 128],
                            ident_bf[hh * 64:(hh + 1) * 64, hh * 64:(hh + 1) * 64])
                        nc.vector.tensor_copy(
                            v_aug[(p, st // 2)][:, st % 2,
                                                hh * 65:hh * 65 + 64], tr[:])
                        yield

        def drain(gen, n):
            if gen is None:
                return
            for _ in range(n):
                try:
                    next(gen)
                except StopIteration:
                    return

        g0 = qkv_gen(0)
        drain(g0, 99)
        for p in range(NPAIR):
            nxt = qkv_gen(p + 1) if p + 1 < NPAIR else None
            drain(nxt, 1)  # kick off the weight DMA
            for qc in range(4):
                qsl = slice(qc * 512, (qc + 1) * 512)
                n_st = 4 * (qc + 1)
                attA = mps.tile([65, 512], f32, tag="attA", name="attA")
                attB = mps.tile([65, 512], f32, tag="attB", name="attB")
                e_q = {}

                def emit_st(si, p=p, qc=qc, qsl=qsl, e_q=e_q):
                    ssl = slice(si * 128, (si + 1) * 128)
                    stAB = mps.tile([P, 2, 512], f32, tag="stAB", bufs=2,
                                    name="stAB")
                    nc.tensor.matmul(stAB[:, 0, :], kk_r[p][0:64, :, ssl],
                                     qq_r[p][0:64, :, qsl],
                                     start=True, stop=True, perf_mode=DR)
                    nc.tensor.matmul(stAB[:, 1, :], kk_r[p][64:128, :, ssl],
                                     qq_r[p][64:128, :, qsl],
                                     start=True, stop=True, perf_mode=DR)
                    if si % 2 == 0:
                        e4 = e_pool.tile([P, 2, 2, 512], fp8, tag="e",
                                         name="e4")
                        e_q[si // 2] = e4
                    else:
                        e4 = e_q[si // 2]
                    epar = e4[:, :, si % 2, :]
                    off = si - 4 * qc
                    if off > 0:
                        # leading 128*off columns are fully masked: zero them
                        # and exp only the live range
                        nc.vector.memset(epar[:, :, 0:off * 128], 0.0)
                        nc.scalar.activation(epar[:, :, off * 128:512],
                                             stAB[:, :, off * 128:512], AF.Exp,
                                             scale=SSCALE)
                    else:
                        nc.scalar.activation(epar[:], stAB[:], AF.Exp,
                                             scale=SSCALE)
                    if off >= 0:
                        dsl = slice(off * 128, off * 128 + 128)
                        nc.vector.tensor_mul(e4[:, 0, si % 2, dsl],
                                             e4[:, 0, si % 2, dsl],
                                             mask_tri8[:, 0:128])
                        nc.vector.tensor_mul(e4[:, 1, si % 2, dsl],
                                             e4[:, 1, si % 2, dsl],
                                             mask_tri8[:, 0:128])

                emit_st(0)
                if n_st > 1:
                    emit_st(1)
                for si in range(n_st):
                    if si + 2 < n_st:
                        emit_st(si + 2)
                    if si % 2 == 1:
                        e4 = e_q.pop(si // 2)
                        va = v_aug[(p, si // 2)]
                        nc.tensor.matmul(attA[:], va[:, :, 0:65],
                                         e4[:, 0, :, :],
                                         start=(si == 1), stop=(si == n_st - 1),
                                         perf_mode=DR)
                        nc.tensor.matmul(attB[:], va[:, :, 65:130],
                                         e4[:, 1, :, :],
                                         start=(si == 1), stop=(si == n_st - 1),
                                         perf_mode=DR)
                    drain(nxt, 1)
                for hh, att in ((0, attA), (1, attB)):
                    rec = rec_pool.tile([1, 512], f32, tag="rec")
                    nc.vector.reciprocal(rec[:], att[64:65, :])
                    bc_sb = bcsb_pool.tile([64, 512], f32, tag="bc_sb")
                    nc.gpsimd.partition_broadcast(bc_sb[:], rec[:])
                    nc.vector.tensor_mul(attT[p][hh * 64:(hh + 1) * 64, qsl],
                                         att[0:64, :], bc_sb[:])
                drain(nxt, 1)
            drain(nxt, 99)
        pat.close()
        px8.close()
        pqkv.close()

        # ===== Phase 4: proj -> ReduceScatter -> x2 (+ fused LN2 stats) =====
        px2bf = ExitStack()
        x28_pool = px2bf.enter_context(tc.tile_pool(name="x28", bufs=1))
        x2h8, x2l8 = {}, {}
        for j in range(NCP):
            for tc2 in range(2):
                x2h8[(j, tc2)] = x28_pool.tile([P, 2, 512], fp8,
                                               name=f"x2h{j}_{tc2}",
                                               tag=f"x2h{j}_{tc2}")
                x2l8[(j, tc2)] = x28_pool.tile([P, 2, 512], fp8,
                                               name=f"x2l{j}_{tc2}",
                                               tag=f"x2l{j}_{tc2}")

        p4 = ExitStack()
        xrpre_pool = p4.enter_context(tc.tile_pool(name="xrpre", bufs=1))
        wp_pool = p4.enter_context(tc.tile_pool(name="wproj", bufs=1))
        proj_ps = p4.enter_context(tc.tile_pool(name="projps", bufs=2, space="PSUM"))
        sa_pool = p4.enter_context(tc.tile_pool(name="sasb", bufs=4))
        xres_pool = p4.enter_context(tc.tile_pool(name="xres", bufs=3))
        bpj_pool = p4.enter_context(tc.tile_pool(name="bpj", bufs=1))
        sq_pool = p4.enter_context(tc.tile_pool(name="sq2", bufs=9))
        rows2_pool = p4.enter_context(tc.tile_pool(name="rows2", bufs=4))
        stat_ps2 = p4.enter_context(tc.tile_pool(name="statps2", bufs=2, space="PSUM"))
        bcp_ps2 = p4.enter_context(tc.tile_pool(name="bcpps2", bufs=1, space="PSUM"))

        bprojt = bpj_pool.tile([P, NCT], f32)
        nc.gpsimd.dma_start(bprojt[:], d_bproj[:])
        bprojcol = [bprojt[:, ci:ci + 1] for ci in range(NCT)]
        xres_t = []
        for co in range(NCT):
            xr_ = xrpre_pool.tile([P, TL], f32, name=f"xres{co}",
                                  tag=f"xres{co}")
            eng = nc.scalar if co % 2 else nc.sync
            eng.dma_start(xr_[:], d_xres[co])
            xres_t.append(xr_)
        wp_t = wp_pool.tile([P, NPAIR, C], bf16)
        nc.scalar.dma_start(wp_t[:], d_wproj[:])
        wp = [wp_t[:, ki] for ki in range(NPAIR)]
        sx_ch = [stat_ps2.tile([32, 512], f32, name=f"sx2_{ch}", tag="sx")
                 for ch in range(2)]
        sq_ch = [stat_ps2.tile([32, 512], f32, name=f"sq2_{ch}", tag="sq")
                 for ch in range(2)]
        x2sq = {}

        def x2_block(co):
            j, i2 = co // 2, co % 2
            sal = xres_pool.tile([P, TL], f32, tag="sal", name="sal")
            if with_collective:
                nc.sync.dma_start(sal[:], sa_local[co // 4][co % 4])
            else:
                nc.sync.dma_start(sal[:], sa_bounce[co // 4][0, co % 4])
            x2sb = xres_pool.tile([P, TL], f32, tag="x2sb", name="x2sb")
            for tc2 in range(2):
                sl2 = slice(tc2 * 512, (tc2 + 1) * 512)
                nc.vector.scalar_tensor_tensor(x2sb[:, sl2], sal[:, sl2],
                                               bprojcol[co][:],
                                               xres_t[co][:, sl2],
                                               OP.add, OP.add)
                nc.vector.tensor_copy(x2h8[(j, tc2)][:, i2, :], x2sb[:, sl2])
                if tc2 == 0:
                    nc.vector.tensor_sub(x2l8[(j, tc2)][:, i2, :], x2sb[:, sl2],
                                         x2h8[(j, tc2)][:, i2, :])
                else:
                    nc.gpsimd.tensor_sub(x2l8[(j, tc2)][:, i2, :], x2sb[:, sl2],
                                         x2h8[(j, tc2)][:, i2, :])
                if i2 == 0:
                    x2sq[(j, tc2)] = sq_pool.tile([P, 2, 512], fp8,
                                                  name="x2sq", tag="sqt")
                nc.scalar.activation(x2sq[(j, tc2)][:, i2, :], x2sb[:, sl2],
                                     AF.Square)
            nc.sync.dma_start(x2_dram[co], x2sb[:])
            if i2 == 1:
                for ch in range(2):
                    nc.tensor.matmul(sx_ch[ch][:], ones8[:], x2h8[(j, ch)][:],
                                     start=(j == 0), stop=(j == NCP - 1),
                                     perf_mode=DR)
                    nc.tensor.matmul(sq_ch[ch][:], ones8[:], x2sq[(j, ch)][:],
                                     start=(j == 0), stop=(j == NCP - 1),
                                     perf_mode=DR)

        for co in range(NCT):
            sa_sb = sa_pool.tile([P, 2, TL], f32, tag="sa_sb")
            for tc4 in range(4):
                sl = slice(tc4 * 512, (tc4 + 1) * 512)
                pp = proj_ps.tile([P, 512], f32, tag="pp")
                for ki in range(NPAIR):
                    nc.tensor.matmul(pp[:], wp[ki][:, co * P:(co + 1) * P],
                                     attT[ki][:, sl],
                                     start=(ki == 0), stop=(ki == NPAIR - 1))
                dst = sa_sb[:, tc4 // 2, (tc4 % 2) * 512:(tc4 % 2) * 512 + 512]
                if tc4 % 2 == 0:
                    nc.vector.tensor_copy(dst, pp[:])
                else:
                    nc.scalar.copy(dst, pp[:])
            for fold in range(2):
                nc.sync.dma_start(sa_bounce[co // 4][fold, co % 4],
                                  sa_sb[:, fold, :])
            if co == 3 or co == NCT - 1:
                h = co // 4
                if with_collective:
                    nc.gpsimd.collective_compute(
                        "ReduceScatter",
                        OP.add,
                        replica_groups=GROUPS,
                        ins=[sa_bounce[h].opt()],
                        outs=[sa_local[h].opt()],
                    )
                # (collective-free twin reads sa_bounce fold 0 directly)
            lag = 4 if with_collective else 1
            if co >= lag:
                x2_block(co - lag)
        for co in range(NCT - (4 if with_collective else 1), NCT):
            x2_block(co)

        mu2 = rows2_pool.tile([1, TL], f32, tag="row")
        ex22 = rows2_pool.tile([1, TL], f32, tag="row")
        for ch in range(2):
            sl = slice(ch * 512, (ch + 1) * 512)
            nc.scalar.mul(mu2[:, sl], sx_ch[ch][0:1, :], 1.0 / C)
            nc.scalar.mul(ex22[:, sl], sq_ch[ch][0:1, :], 1.0 / C)
        var2 = rows2_pool.tile([1, TL], f32, tag="row")
        nc.vector.tensor_mul(var2[:], mu2[:], mu2[:])
        nc.vector.scalar_tensor_tensor(var2[:], ex22[:], EPS,
                                       var2[:], OP.add, OP.subtract)
        std2 = rows2_pool.tile([1, TL], f32, tag="row")
        nc.scalar.activation(std2[:], var2[:], AF.Sqrt)
        rstd2 = rows2_pool.tile([1, TL], f32, tag="row")
        nc.vector.reciprocal(rstd2[:], std2[:])
        nm2 = rows2_pool.tile([1, TL], f32, tag="row")
        nc.scalar.mul(nm2[:], mu2[:], -1.0)

        xrow2_8 = rowr_pool.tile([1, 2, TL], fp8, tag="xrow2_8")
        nc.vector.tensor_copy(xrow2_8[0:1, 0, :], nm2[:])
        nc.vector.tensor_copy(xrow2_8[0:1, 1, :], std2[:])
        for ch in range(2):
            sl = slice(ch * 512, (ch + 1) * 512)
            bc_ps = bcp_ps2.tile([P, 512], f32, tag="bc")
            nc.tensor.matmul(bc_ps[:], onescol[:], rstd2[:, sl],
                             start=True, stop=True)
            nc.scalar.mul(a2bc[ch][:], bc_ps[:], 1.0 / (WSCALE * WSCALE))
        p4.close()

        # ====== Phase 6: FFN (LN folded, rstd2 deferred to ff2 evict) ======
        # 3-term fp8 residual GEMMs: Wh@xh + Wl@xh + Wh@xl, DoubleRow mode.
        p6 = ExitStack()
        w1_pool = p6.enter_context(tc.tile_pool(name="w1", bufs=3))
        ccf_pool = p6.enter_context(tc.tile_pool(name="ccf", bufs=1))
        w2_pool = p6.enter_context(tc.tile_pool(name="w2", bufs=3))
        ffn_ps = p6.enter_context(tc.tile_pool(name="ffnps", bufs=3, space="PSUM"))
        rlbf_pool = p6.enter_context(tc.tile_pool(name="rlbf", bufs=4))
        relu_pool = p6.enter_context(tc.tile_pool(name="relu", bufs=1))
        out_pool = p6.enter_context(tc.tile_pool(name="outsb", bufs=2))
        b2_pool = p6.enter_context(tc.tile_pool(name="b2p", bufs=1))

        b2t = b2_pool.tile([P, NCT], f32)
        nc.gpsimd.dma_start(b2t[:], d_b2[:])
        b2col = [b2t[:, ci:ci + 1] for ci in range(NCT)]
        ccf_all = ccf_pool.tile([1, 8, 2, 512], fp8)
        nc.gpsimd.dma_start(ccf_all[:], d_ccf[:].unsqueeze(0))

        rl_h8, rl_l8 = {}, {}
        for q2 in range(NFP):
            for tc2 in range(2):
                rl_h8[(q2, tc2)] = relu_pool.tile([P, 2, 512], fp8,
                                                  name=f"rlh{q2}_{tc2}",
                                                  tag=f"rlh{q2}_{tc2}")
                rl_l8[(q2, tc2)] = relu_pool.tile([P, 2, 512], fp8,
                                                  name=f"rll{q2}_{tc2}",
                                                  tag=f"rll{q2}_{tc2}")

        for fog in range(8):
            w1t = w1_pool.tile([P, NCP, 2, 1024], fp8, tag="w1t")
            nc.scalar.dma_start(w1t[:], d_w1hl[fog])
            w1h_t = [w1t[:, j, :, 0:512] for j in range(NCP)]
            w1l_t = [w1t[:, j, :, 512:1024] for j in range(NCP)]
            ccf = ccf_all[:, fog]
            for fol in range(4):
                fo = fog * 4 + fol
                fsl = slice(fol * P, (fol + 1) * P)
                q2, i2 = fo // 2, fo % 2
                for tc2 in range(2):
                    sl = slice(tc2 * 512, (tc2 + 1) * 512)
                    fp = ffn_ps.tile([P, 512], f32, tag="fp")
                    for j in range(NCP):
                        nc.tensor.matmul(fp[:], w1h_t[j][:, :, fsl],
                                         x2h8[(j, tc2)][:],
                                         start=(j == 0), stop=False, perf_mode=DR)
                    for j in range(NCP):
                        nc.tensor.matmul(fp[:], w1l_t[j][:, :, fsl],
                                         x2h8[(j, tc2)][:],
                                         start=False, stop=False, perf_mode=DR)
                    for j in range(NCP):
                        nc.tensor.matmul(fp[:], w1h_t[j][:, :, fsl],
                                         x2l8[(j, tc2)][:],
                                         start=False, stop=False, perf_mode=DR)
                    nc.tensor.matmul(fp[:], ccf[:, :, fsl], xrow2_8[:, :, sl],
                                     start=False, stop=True, perf_mode=DR)
                    rlbf = rlbf_pool.tile([P, 512], bf16, tag="rlbf")
                    nc.scalar.activation(rlbf[:], fp[:], AF.Relu)
                    nc.vector.tensor_copy(rl_h8[(q2, tc2)][:, i2, :], rlbf[:])
                    nc.vector.tensor_sub(rl_l8[(q2, tc2)][:, i2, :], rlbf[:],
                                         rl_h8[(q2, tc2)][:, i2, :])

        for co in range(NCT):
            w2t = w2_pool.tile([P, NFP, 2, 256], fp8, tag="w2t")
            nc.scalar.dma_start(w2t[:], d_w2hl[co])
            x2c = out_pool.tile([P, TL], f32, tag="x2c")
            nc.sync.dma_start(x2c[:], x2_dram[co])
            osb = out_pool.tile([P, TL], f32, tag="osb")
            for tc2 in range(2):
                sl = slice(tc2 * 512, (tc2 + 1) * 512)
                fp = ffn_ps.tile([P, 512], f32, tag="fp")
                for q2 in range(NFP):
                    nc.tensor.matmul(fp[:], w2t[:, q2, :, 0:P], rl_h8[(q2, tc2)][:],
                                     start=(q2 == 0), stop=False, perf_mode=DR)
                for q2 in range(NFP):
                    nc.tensor.matmul(fp[:], w2t[:, q2, :, P:2 * P],
                                     rl_h8[(q2, tc2)][:],
                                     start=False, stop=False, perf_mode=DR)
                for q2 in range(NFP):
                    nc.tensor.matmul(fp[:], w2t[:, q2, :, 0:P], rl_l8[(q2, tc2)][:],
                                     start=False, stop=(q2 == NFP - 1),
                                     perf_mode=DR)
                tmp = out_pool.tile([P, 512], f32, tag="tmp")
                nc.vector.tensor_mul(tmp[:], fp[:], a2bc[tc2][:])
                nc.vector.scalar_tensor_tensor(osb[:, sl], tmp[:], b2col[co][:],
                                               x2c[:, sl], OP.add, OP.add)
            nc.sync.dma_start(d_out[co], osb[:])
        p6.close()
        px2bf.close()
        pattr.close()

    nc.compile()
    return nc


def _fp8_hi_lo(w):
    hi = w.astype(F8)
    lo = (w - hi.astype(np.float32)).astype(F8)
    return hi, lo


def _dr_fold(w, inner):
    """[C, N] -> [C//256, 128, 2, N] DoubleRow fold over c-tile pairs."""
    Cdim = w.shape[0]
    return np.ascontiguousarray(
        w.reshape(Cdim // 256, 2, P, *inner).swapaxes(1, 2))


def _prep_inputs(x, Wq, Wk, Wv, Wproj, bproj, W1, b1, W2, b2, g1, beta1, g2, beta2):
    """Build the 8 per-core input maps (host-side sharding + layout prep)."""
    f32 = np.float32
    scale = HS ** -0.5
    x = np.asarray(x, f32)
    Wq = np.asarray(Wq, f32)
    Wk = np.asarray(Wk, f32)
    Wv = np.asarray(Wv, f32)
    Wproj = np.asarray(Wproj, f32)
    W1 = np.asarray(W1, f32)
    b1 = np.asarray(b1, f32)
    W2 = np.asarray(W2, f32)
    g1 = np.asarray(g1, f32)
    beta1 = np.asarray(beta1, f32)
    g2 = np.asarray(g2, f32)
    beta2 = np.asarray(beta2, f32)

    w1g = WSCALE * g2[:, None] * W1            # [C, F] (fp8-range scaled)
    b1p = WSCALE * (b1 + beta2 @ W1)
    w1h, w1l = _fp8_hi_lo(w1g)
    # [8, P, NCP, 2, 512] each: DR fold over c, chunk f into 8 groups of 512
    def fold_w1(w):
        return np.ascontiguousarray(
            _dr_fold(w.astype(f32), (F,)).astype(F8).reshape(
                NCP, P, 2, 8, 512).transpose(3, 1, 0, 2, 4))
    # merged hi|lo along the last axis: [8, P, NCP, 2, 1024]
    w1hl_d = np.concatenate([fold_w1(w1h), fold_w1(w1l)], axis=4)
    # corrections [8, 2, 512]: row 0 = colsum(w1g) (x -mu), row 1 = b1p (x std)
    ccf = np.stack([w1g.sum(0).reshape(8, 512),
                    b1p.reshape(8, 512)], axis=1).astype(F8)
    # W2 [F, C] -> hi/lo -> [NCT, NFP, P, 2, P] each, merged -> [...,2,256]
    w2h, w2l = _fp8_hi_lo(WSCALE * W2)

    def fold_w2(w):
        # [F, C] -> [NFP, 2, P, NCT, P] -> [NCT, P, NFP, 2, P]
        return np.ascontiguousarray(
            w.astype(f32).reshape(NFP, 2, P, NCT, P).transpose(3, 2, 0, 1, 4)
        ).astype(F8)

    w2hl_d = np.concatenate([fold_w2(w2h), fold_w2(w2l)], axis=4)
    b2r = np.ascontiguousarray(np.asarray(b2, f32).reshape(NCT, P).T)
    bprojr = np.ascontiguousarray(np.asarray(bproj, f32).reshape(NCT, P).T)
    sp = np.arange(P)[:, None]
    qf = np.arange(512)[None, :]
    m01 = np.ascontiguousarray(np.stack(
        [(sp + 128 * off <= qf) for off in range(4)]).transpose(1, 0, 2)).astype(
        ml_dtypes.bfloat16)
    ident = np.eye(P, dtype=f32)

    xT = [np.ascontiguousarray(
        x[b].T.reshape(NCT, P, T).transpose(1, 0, 2)) for b in range(B)]

    def pair_weights(Wfull, g, scl):
        # raw pair weights [NPAIR, C, P]: cols 0:64 head g*8+2p, 64:128 head +1
        out = np.empty((NPAIR, C, P), f32)
        for p in range(NPAIR):
            hA, hB = g * 8 + 2 * p, g * 8 + 2 * p + 1
            out[p, :, 0:64] = Wfull[hA] * scl
            out[p, :, 64:128] = Wfull[hB] * scl
        return out

    per_g = {}
    for g in range(2):
        d = {}
        wqkv = np.empty((NPAIR, C, 3 * P), f32)
        ccqkv = np.empty((NPAIR, 2, 3 * P), f32)
        for jj, (Wfull, scl) in enumerate(((Wq, WSCALE), (Wk, WSCALE),
                                           (Wv, WSCALE))):
            raw = pair_weights(Wfull, g, scl)      # [NPAIR, C, P]
            wt = g1[None, :, None] * raw           # g1-folded
            wqkv[:, :, jj * P:(jj + 1) * P] = wt
            ccqkv[:, 0, jj * P:(jj + 1) * P] = wt.sum(1)
            ccqkv[:, 1, jj * P:(jj + 1) * P] = np.einsum("c,pcd->pd", beta1, raw)
        # DR fold: [NPAIR, P, NCP, 2, 3P]
        d["wqkv"] = np.ascontiguousarray(
            wqkv.reshape(NPAIR, NCP, 2, P, 3 * P).transpose(0, 3, 1, 2, 4)
        ).astype(F8)
        d["ccqkv"] = ccqkv[:, None].astype(F8)
        d["wproj"] = np.ascontiguousarray(
            Wproj[g * 512:(g + 1) * 512].reshape(NPAIR, P, C).transpose(
                1, 0, 2)).astype(ml_dtypes.bfloat16)
        per_g[g] = d

    in_maps = []
    for c in range(8):
        b, g = c // 2, c % 2
        m = {
            "xT": xT[b],
            "xresT": np.ascontiguousarray(
                xT[b].transpose(1, 0, 2)[:, :, g * TL:(g + 1) * TL]),
            "bproj": bprojr,
            "w1hl": w1hl_d,
            "ccf": ccf,
            "w2hl": w2hl_d,
            "b2": b2r,
            "m01": m01,
            "ident": ident,
        }
        m.update(per_g[g])
        in_maps.append(m)
    return in_maps


def kernel(**inputs):
    from concourse.bass_utils import run_bass_kernel_spmd

    if "nc" not in _CACHE:
        _CACHE["nc"] = _build(with_collective=True)
    nc = _CACHE["nc"]
    in_maps = _prep_inputs(**inputs)
    res = None
    last_err = None
    for _attempt in range(3):
        try:
            res = run_bass_kernel_spmd(nc, in_maps, list(range(8)))
            break
        except Exception as e:  # transient runtime/tunnel hiccups
            last_err = e
            import time
            time.sleep(10)
    if res is None:
        raise last_err
    out = np.empty((B, T, C), np.float32)
    for c in range(8):
        b, g = c // 2, c % 2
        outT = res.results[c]["outT"].reshape(C, TL)
        out[b, g * TL:(g + 1) * TL, :] = outT.T
    return out
